# revision 2
# baseline (speedup 1.0000x reference)
"""Trainium2 Bass kernel for nn_MultiHeadAttention (B=2, S=2048, D=1024, H=16).

Sharding: 8 cores = 2 batch groups x 4 head-groups (4 heads/core).
Per core: QKV projections (f32r matmuls), RoPE (DVE), causal attention
(scoresT = K^T-layout matmuls + batched exp on ACT + PV with fused rowsum via
[V|ones] stationary), output projection partials; host sums the 4 partials
per batch and transposes back.

Self-contained: hardcodes shapes; only imports concourse/numpy.
"""
import numpy as np
from contextlib import ExitStack

import concourse.bass as bass
import concourse.bacc as bacc
import concourse.mybir as mybir
import concourse.tile as tile
from concourse.tile import add_dep_helper
from concourse.bass_utils import run_bass_kernel_spmd

F32 = mybir.dt.float32
F32R = mybir.dt.float32r
BF16 = mybir.dt.bfloat16
AF = mybir.ActivationFunctionType
OP = mybir.AluOpType

B, S, D, H = 2, 2048, 1024, 16
HD = 64          # head dim
HPC = 4          # heads per core
N_CORES = 8
SC = 512         # s-chunk for projections / i-tile width
NSC = S // SC    # 4
NDB = D // 128   # 8 d-blocks
NJB = S // 128   # 16 j-blocks
DLOC = HPC * HD  # 256 local channels


def preround(x):
    """Round fp32 to f32r (round-half-up on low 12 mantissa bits)."""
    xb = np.ascontiguousarray(x, np.float32).view(np.uint32)
    return (((xb.astype(np.uint64) + 0x800) & 0xFFFFF000)
            .astype(np.uint32)).view(np.float32)


def build_program(niter=1, phases='paw', adet=3):
    """Build + compile the per-core SPMD program. niter>1 wraps the whole
    kernel in a device-side loop (for timing)."""
    nc = bacc.Bacc("TRN2", target_bir_lowering=False, debug=False,
                   num_devices=N_CORES)

    d_xt = nc.dram_tensor("xt", [D, S], F32R, kind="ExternalInput").ap()
    d_wq = nc.dram_tensor("wq", [D, DLOC], F32R, kind="ExternalInput").ap()
    d_wk = nc.dram_tensor("wk", [D, DLOC], F32R, kind="ExternalInput").ap()
    d_wv = nc.dram_tensor("wv", [D, DLOC], F32R, kind="ExternalInput").ap()
    d_wo = nc.dram_tensor("wo", [DLOC, D], F32R, kind="ExternalInput").ap()
    d_cos = nc.dram_tensor("cosT", [128, S], F32, kind="ExternalInput").ap()
    d_sin = nc.dram_tensor("sinT", [128, S], F32, kind="ExternalInput").ap()
    d_ones = nc.dram_tensor("ones", [128, NJB * HPC * 64], F32R,
                            kind="ExternalInput").ap()
    d_mask = nc.dram_tensor("masktri", [128, 128], F32,
                            kind="ExternalInput").ap()
    d_out = nc.dram_tensor("outp", [D, S], F32, kind="ExternalOutput").ap()

    with tile.TileContext(nc) as tc:
        ctx = ExitStack()
        sb = ctx.enter_context(tc.tile_pool(name="sb", bufs=1))
        ps = ctx.enter_context(tc.tile_pool(name="ps", bufs=1, space="PSUM"))

        _last_pe = [None]

        def _pe_break():
            _last_pe[0] = None

        def _pe(bi):
            if _last_pe[0] is not None:
                add_dep_helper(bi.ins, _last_pe[0], sync=False,
                               reason="pe-order")
            _last_pe[0] = bi.ins
            return bi

        # persistent inputs (loaded once, outside the timing loop)
        wq = sb.tile([128, NDB, DLOC], F32R, tag="wq")
        wk = sb.tile([128, NDB, DLOC], F32R, tag="wk")
        wv = sb.tile([128, NDB, DLOC], F32R, tag="wv")
        wo = sb.tile([128, 2, D], F32R, tag="wo")
        cosT = sb.tile([128, S], F32, tag="cs0")
        sinT = sb.tile([128, S], F32, tag="cs1")
        mtri = sb.tile([128, 128], F32, tag="mt")
        nc.gpsimd.dma_start(wq[:], d_wq[:].rearrange("(b p) c -> p b c", b=NDB))
        nc.gpsimd.dma_start(wk[:], d_wk[:].rearrange("(b p) c -> p b c", b=NDB))
        nc.gpsimd.dma_start(wv[:], d_wv[:].rearrange("(b p) c -> p b c", b=NDB))
        nc.gpsimd.dma_start(wo[:], d_wo[:].rearrange("(b p) c -> p b c", b=2))
        nc.gpsimd.dma_start(cosT[:], d_cos[:])
        nc.gpsimd.dma_start(sinT[:], d_sin[:])
        nc.gpsimd.dma_start(mtri[:], d_mask[:])
        mtri2 = sb.tile([128, 2, 128], F32, tag="mt2")
        nc.gpsimd.dma_start(mtri2[:, 0, :], d_mask[:])
        nc.gpsimd.dma_start(mtri2[:, 1, :], d_mask[:])
        # V' = [s, jb, head, V(64)|ones(64)] -- persistent; ones loaded once
        vp = sb.tile([128, NJB, HPC, 128], F32R, tag="vp")
        nc.gpsimd.dma_start(
            vp[:, :, :, 64:128],
            d_ones[:].rearrange("p (j h c) -> p j h c", j=NJB, h=HPC))

        # rope'd Q/K in [channel, s] layout; evens/odds split tiles.
        # rows of qe: [h0 ch(0,2,..62) -> 0..31 | h1 -> 32..63 | ...]
        qe = sb.tile([128, S], BF16, tag="qe")
        qo = sb.tile([128, S], BF16, tag="qo")
        ke = sb.tile([128, S], BF16, tag="ke")
        ko = sb.tile([128, S], BF16, tag="ko")
        cx0 = sb.tile([128, S], F32R, tag="cx0")  # ctxT heads 0,1
        cx1 = sb.tile([128, S], F32R, tag="cx1")  # ctxT heads 2,3

        def body(_i=None):

            def proj_chunk(sc):
                s0 = sc * SC
                xt = sb.tile([128, NDB, SC], F32R, tag="xt", bufs=2, name="xt")
                eng = nc.sync if sc % 2 == 0 else nc.gpsimd
                eng.dma_start(
                    xt[:],
                    d_xt[:].rearrange("(b p) s -> p b s", b=NDB)[:, :, s0:s0 + SC])
                for wi, (w, te, to) in enumerate(((wq, qe, qo), (wk, ke, ko))):
                    pp = ps.tile([128, 2, SC], F32, tag="st", bufs=2,
                                 name="pp")
                    pps = [pp[:, 0, :], pp[:, 1, :]]
                    for half in range(2):
                        for db in range(NDB):
                            nc.tensor.matmul(
                                pps[half][:],
                                w[:, db, 128 * half:128 * half + 128],
                                xt[:, db, :],
                                start=(db == 0), stop=(db == NDB - 1))
                    t1 = sb.tile([128, SC], F32, tag="t0", bufs=2, name="t1")
                    t2 = sb.tile([128, SC], F32, tag="t1", bufs=2, name="t2")
                    cs = cosT[:, s0:s0 + SC]
                    sn = sinT[:, s0:s0 + SC]
                    nc.vector.tensor_tensor(t1[:], pps[0][:], cs, OP.mult)
                    nc.vector.tensor_tensor(t2[:], pps[1][:], sn, OP.mult)
                    nc.vector.tensor_tensor(te[:, s0:s0 + SC], t1[:], t2[:],
                                            OP.subtract)
                    t3 = sb.tile([128, SC], F32, tag="t0", bufs=2, name="t3")
                    t4 = sb.tile([128, SC], F32, tag="t1", bufs=2, name="t4")
                    nc.vector.tensor_tensor(t3[:], pps[1][:], cs, OP.mult)
                    nc.vector.tensor_tensor(t4[:], pps[0][:], sn, OP.mult)
                    nc.vector.tensor_tensor(to[:, s0:s0 + SC], t3[:], t4[:],
                                            OP.add)
                for sb4 in range(SC // 128):
                    jb = sc * (SC // 128) + sb4
                    vps = ps.tile([128, DLOC], F32, tag="pv0", name="vps")
                    for db in range(NDB):
                        nc.tensor.matmul(
                            vps[:], xt[:, db, 128 * sb4:128 * sb4 + 128],
                            wv[:, db, :],
                            start=(db == 0), stop=(db == NDB - 1))
                    nc.vector.tensor_copy(
                        vp[:, jb, :, 0:64],
                        vps[:].rearrange("p (h c) -> p h c", h=HPC))

            def attn_itile(it):
                i0 = it * SC
                njb = i0 // 128 + 4
                pvs = [ps.tile([128, SC], F32, tag=f"pv{h}", name=f"pvt{h}")
                       for h in range(HPC)]
                pending = []
                for jb in range(njb):
                    _pe_break()
                    j0 = jb * 128
                    off = max(0, j0 - i0)
                    for hp in range(2):  # head pairs (0,1) and (2,3)
                        ss = ps.tile([128, 2, SC], F32, tag="st", bufs=2,
                                     name="ssjb")
                        for hh in range(2):
                            h = 2 * hp + hh
                            r0 = 32 * h
                            kesl = ke[r0:r0 + 32, j0:j0 + 128]
                            kosl = ko[r0:r0 + 32, j0:j0 + 128]
                            _pe(nc.tensor.ldweights(kesl,
                                                    tile_position=(r0, 0)))
                            _pe(nc.tensor.matmul(
                                ss[:, hh, off:SC], kesl,
                                qe[r0:r0 + 32, i0 + off:i0 + SC],
                                start=True, stop=False, tile_position=(r0, 0)))
                            _pe(nc.tensor.ldweights(kosl,
                                                    tile_position=(r0, 0)))
                            _pe(nc.tensor.matmul(
                                ss[:, hh, off:SC], kosl,
                                qo[r0:r0 + 32, i0 + off:i0 + SC],
                                start=False, stop=True, tile_position=(r0, 0)))
                        pt = sb.tile([128, 2, SC], F32R, tag="pt", bufs=4,
                                     name="pt")
                        nc.scalar.activation(pt[:, :, off:SC], ss[:, :, off:SC],
                                             AF.Exp, scale=0.125)
                        if j0 >= i0 and adet >= 3:
                            # diagonal: mask both heads' [128,128] windows
                            nc.vector.tensor_tensor(
                                pt[:, :, off:off + 128],
                                pt[:, :, off:off + 128], mtri2[:], OP.mult)
                        if adet >= 2:
                            def mk_pv(jb=jb, hp=hp, pt=pt, off=off):
                                for hh in range(2):
                                    h = 2 * hp + hh
                                    nc.tensor.matmul(
                                        pvs[h][:, off:SC],
                                        vp[:, jb, h, :], pt[:, hh, off:SC],
                                        start=(jb == 0),
                                        stop=(jb == njb - 1))
                            pending.append(mk_pv)
                    # emit the PREVIOUS j-block's PV after this block's scores
                    # so the PE stream never stalls on the current exp
                    while len(pending) > 2:
                        pending.pop(0)()
                for fn in pending:
                    fn()
                # normalize: ctx = pv[0:64] / r (r = pv[64:128], replicated)
                for h in range(HPC if adet >= 3 else 0):
                    rt = sb.tile([64, SC], F32, tag="t0", bufs=2, name="rt")
                    nc.scalar.activation(rt[:], pvs[h][64:128, :], AF.Copy)
                    rr = sb.tile([64, SC], F32, tag="t1", bufs=2, name="rr")
                    nc.vector.reciprocal_approx_fast(rr[:], rt[:])
                    cx = (cx0, cx1)[h // 2]
                    nc.vector.tensor_tensor(
                        cx[64 * (h % 2):64 * (h % 2) + 64, i0:i0 + SC],
                        pvs[h][0:64, :], rr[:], OP.mult)

            def wo_itile(it):
                i0 = it * SC
                for dt in range(NDB):
                    ops_ = ps.tile([128, SC], F32, tag="pv1", name="ops")
                    for dcb, cx in enumerate((cx0, cx1)):
                        nc.tensor.matmul(
                            ops_[:], wo[:, dcb, dt * 128:dt * 128 + 128],
                            cx[:, i0:i0 + SC],
                            start=(dcb == 0), stop=(dcb == 1))
                    ost = sb.tile([128, SC], F32, tag="ost", bufs=3, name="ost")
                    nc.vector.tensor_copy(ost[:], ops_[:])
                    oeng = (nc.sync, nc.sync, nc.gpsimd)[dt % 3]
                    oeng.dma_start(
                        d_out[dt * 128:(dt + 1) * 128, i0:i0 + SC], ost[:])

            w_ = "w" in phases
            if "p" in phases and "a" in phases:
                proj_chunk(0)
                proj_chunk(1)
                attn_itile(0)
                proj_chunk(2)
                if w_: wo_itile(0)
                attn_itile(1)
                proj_chunk(3)
                if w_: wo_itile(1)
                attn_itile(2)
                if w_: wo_itile(2)
                attn_itile(3)
                if w_: wo_itile(3)
            elif "p" in phases:
                for sc_ in range(NSC):
                    proj_chunk(sc_)
            elif "a" in phases:
                for it_ in range(NSC):
                    attn_itile(it_)
                if w_:
                    for it_ in range(NSC):
                        wo_itile(it_)
            elif w_:
                for it_ in range(NSC):
                    wo_itile(it_)

        if niter == 1:
            body()
        else:
            if "p" not in phases:
                # populate qe/ke/vp/cx once so the timed loop has real data
                saved = phases
                phases = "pa"
                body()
                phases = saved
            with tc.For_i(0, niter, 1) as i:
                body(i)
        ctx.close()
    nc.compile()
    return nc


def prep_inputs(x, pos_cos, pos_sin, Wq, Wk, Wv, Wo):
    """Host-side prep: per-core input dicts."""
    cosT = np.ascontiguousarray(np.tile(pos_cos.T, (4, 1)), np.float32)
    sinT = np.ascontiguousarray(np.tile(pos_sin.T, (4, 1)), np.float32)
    ones = np.ones((128, NJB * HPC * 64), np.float32)
    masktri = np.triu(np.ones((128, 128), np.float32))
    in_maps = []
    for c in range(N_CORES):
        b, g = c // 4, c % 4
        heads = [4 * g + h for h in range(HPC)]
        ecols = np.concatenate([64 * h + np.arange(0, 64, 2) for h in heads])
        ocols = np.concatenate([64 * h + np.arange(1, 64, 2) for h in heads])
        perm = np.concatenate([ecols, ocols])
        vcols = np.concatenate([64 * h + np.arange(64) for h in heads])
        in_maps.append({
            "xt": preround(x[b].T),
            "wq": preround(Wq[:, perm]),
            "wk": preround(Wk[:, perm]),
            "wv": preround(Wv[:, vcols]),
            "wo": preround(Wo[vcols, :]),
            "cosT": cosT, "sinT": sinT, "ones": ones, "masktri": masktri,
        })
    return in_maps


_NC_CACHE = {}


def get_program(niter=1, phases="paw", adet=3):
    key = (niter, phases, adet)
    if key not in _NC_CACHE:
        _NC_CACHE[key] = build_program(niter, phases, adet)
    return _NC_CACHE[key]


def run_on_cores(nc, in_maps, **kw):
    return run_bass_kernel_spmd(nc, in_maps, list(range(N_CORES)), **kw)


def kernel(x, pos_cos, pos_sin, Wq, Wk, Wv, Wo, bo):
    nc = get_program(1)
    in_maps = prep_inputs(x, pos_cos, pos_sin, Wq, Wk, Wv, Wo)
    res = run_on_cores(nc, in_maps)
    out = np.empty((B, S, D), np.float32)
    for b in range(B):
        acc = res.results[4 * b]["outp"].astype(np.float64)
        for g in range(1, 4):
            acc += res.results[4 * b + g]["outp"]
        out[b] = (acc.T + np.asarray(bo, np.float64)[None, :]).astype(np.float32)
    return out



# revision 3
# speedup vs baseline: 1.4175x; 1.4175x over previous
"""Trainium2 Bass kernel for nn_MultiHeadAttention (B=2, S=2048, D=1024, H=16).

Sharding: 8 cores = 2 batch groups x 4 head-groups (4 heads/core).
Per core: QKV projections (f32r matmuls), RoPE (DVE), causal attention
(scoresT = K^T-layout matmuls + batched exp on ACT + PV with fused rowsum via
[V|ones] stationary), output projection partials; host sums the 4 partials
per batch and transposes back.

Self-contained: hardcodes shapes; only imports concourse/numpy.
"""
import numpy as np
import ml_dtypes
from contextlib import ExitStack

import concourse.bass as bass
import concourse.bacc as bacc
import concourse.mybir as mybir
import concourse.tile as tile
from concourse.tile import add_dep_helper
from concourse.bass_utils import run_bass_kernel_spmd

F32 = mybir.dt.float32
F32R = mybir.dt.float32r
BF16 = mybir.dt.bfloat16
AF = mybir.ActivationFunctionType
OP = mybir.AluOpType

B, S, D, H = 2, 2048, 1024, 16
HD = 64          # head dim
HPC = 4          # heads per core
N_CORES = 8
SC = 512         # s-chunk for projections / i-tile width
NSC = S // SC    # 4
NDB = D // 128   # 8 d-blocks
NJB = S // 128   # 16 j-blocks
DLOC = HPC * HD  # 256 local channels


def tobf16(x):
    return np.ascontiguousarray(x, np.float32).astype(ml_dtypes.bfloat16)


def preround(x):
    """Round fp32 to f32r (round-half-up on low 12 mantissa bits)."""
    xb = np.ascontiguousarray(x, np.float32).view(np.uint32)
    return (((xb.astype(np.uint64) + 0x800) & 0xFFFFF000)
            .astype(np.uint32)).view(np.float32)


def build_program(niter=1, phases='paw', adet=3):
    """Build + compile the per-core SPMD program. niter>1 wraps the whole
    kernel in a device-side loop (for timing)."""
    nc = bacc.Bacc("TRN2", target_bir_lowering=False, debug=False,
                   num_devices=N_CORES)

    d_xt = nc.dram_tensor("xt", [D, S], BF16, kind="ExternalInput").ap()
    d_wq = nc.dram_tensor("wq", [D, DLOC], BF16, kind="ExternalInput").ap()
    d_wk = nc.dram_tensor("wk", [D, DLOC], BF16, kind="ExternalInput").ap()
    d_wv = nc.dram_tensor("wv", [D, DLOC], BF16, kind="ExternalInput").ap()
    d_wo = nc.dram_tensor("wo", [DLOC, D], F32R, kind="ExternalInput").ap()
    d_cos = nc.dram_tensor("cosT", [128, S], F32, kind="ExternalInput").ap()
    d_sin = nc.dram_tensor("sinT", [128, S], F32, kind="ExternalInput").ap()
    d_ones = nc.dram_tensor("ones", [128, NJB * HPC * 64], F32R,
                            kind="ExternalInput").ap()
    d_mask = nc.dram_tensor("masktri", [128, 128], F32,
                            kind="ExternalInput").ap()
    d_out = nc.dram_tensor("outp", [D, S], F32, kind="ExternalOutput").ap()

    with tile.TileContext(nc) as tc:
        ctx = ExitStack()
        sb = ctx.enter_context(tc.tile_pool(name="sb", bufs=1))
        ps = ctx.enter_context(tc.tile_pool(name="ps", bufs=1, space="PSUM"))

        _last_pe = [None]

        def _pe_break():
            _last_pe[0] = None

        def _pe(bi):
            if _last_pe[0] is not None:
                add_dep_helper(bi.ins, _last_pe[0], sync=False,
                               reason="pe-order")
            _last_pe[0] = bi.ins
            return bi

        # persistent inputs (loaded once, outside the timing loop)
        wq = sb.tile([128, NDB, DLOC], BF16, tag="wq")
        wk = sb.tile([128, NDB, DLOC], BF16, tag="wk")
        wv = sb.tile([128, NDB, DLOC], BF16, tag="wv")
        wo = sb.tile([128, 2, D], F32R, tag="wo")
        cosT = sb.tile([128, S], F32, tag="cs0")
        sinT = sb.tile([128, S], F32, tag="cs1")
        mtri = sb.tile([128, 128], F32, tag="mt")
        nc.gpsimd.dma_start(wq[:], d_wq[:].rearrange("(b p) c -> p b c", b=NDB))
        nc.gpsimd.dma_start(wk[:], d_wk[:].rearrange("(b p) c -> p b c", b=NDB))
        nc.gpsimd.dma_start(wv[:], d_wv[:].rearrange("(b p) c -> p b c", b=NDB))
        nc.gpsimd.dma_start(wo[:], d_wo[:].rearrange("(b p) c -> p b c", b=2))
        nc.gpsimd.dma_start(cosT[:], d_cos[:])
        nc.gpsimd.dma_start(sinT[:], d_sin[:])
        nc.gpsimd.dma_start(mtri[:], d_mask[:])
        mtri2 = sb.tile([128, 2, 128], F32, tag="mt2")
        nc.gpsimd.dma_start(mtri2[:, 0, :], d_mask[:])
        nc.gpsimd.dma_start(mtri2[:, 1, :], d_mask[:])
        # V' = [s, jb, head, V(64)|ones(64)] -- persistent; ones loaded once
        vp = sb.tile([128, NJB, HPC, 128], F32R, tag="vp")
        nc.gpsimd.dma_start(
            vp[:, :, :, 64:128],
            d_ones[:].rearrange("p (j h c) -> p j h c", j=NJB, h=HPC))

        # rope'd Q/K in [channel, s] layout; evens/odds split tiles.
        # rows of qe: [h0 ch(0,2,..62) -> 0..31 | h1 -> 32..63 | ...]
        qe = sb.tile([128, S], BF16, tag="qe")
        qo = sb.tile([128, S], BF16, tag="qo")
        ke = sb.tile([128, S], BF16, tag="ke")
        ko = sb.tile([128, S], BF16, tag="ko")
        cx0 = sb.tile([128, S], F32R, tag="cx0")  # ctxT heads 0,1
        cx1 = sb.tile([128, S], F32R, tag="cx1")  # ctxT heads 2,3

        def body(_i=None):

            def proj_chunk(sc):
                s0 = sc * SC
                xt = sb.tile([128, NDB, SC], BF16, tag="xt", bufs=2, name="xt")
                eng = nc.sync if sc % 2 == 0 else nc.gpsimd
                eng.dma_start(
                    xt[:],
                    d_xt[:].rearrange("(b p) s -> p b s", b=NDB)[:, :, s0:s0 + SC])
                for wi, (w, te, to) in enumerate(((wq, qe, qo), (wk, ke, ko))):
                    pp = ps.tile([128, 2, SC], F32, tag="st", bufs=2,
                                 name="pp")
                    pps = [pp[:, 0, :], pp[:, 1, :]]
                    for half in range(2):
                        _pe_break()
                        for db in range(NDB):
                            wsl = w[:, db, 128 * half:128 * half + 128]
                            _pe(nc.tensor.ldweights(wsl))
                            _pe(nc.tensor.matmul(
                                pps[half][:], wsl,
                                xt[:, db, :],
                                start=(db == 0), stop=(db == NDB - 1)))
                    t1 = sb.tile([128, SC], F32, tag="t0", bufs=2, name="t1")
                    t2 = sb.tile([128, SC], F32, tag="t1", bufs=2, name="t2")
                    cs = cosT[:, s0:s0 + SC]
                    sn = sinT[:, s0:s0 + SC]
                    nc.vector.tensor_tensor(t1[:], pps[0][:], cs, OP.mult)
                    nc.vector.tensor_tensor(t2[:], pps[1][:], sn, OP.mult)
                    nc.vector.tensor_tensor(te[:, s0:s0 + SC], t1[:], t2[:],
                                            OP.subtract)
                    t3 = sb.tile([128, SC], F32, tag="t0", bufs=2, name="t3")
                    t4 = sb.tile([128, SC], F32, tag="t1", bufs=2, name="t4")
                    nc.vector.tensor_tensor(t3[:], pps[1][:], cs, OP.mult)
                    nc.vector.tensor_tensor(t4[:], pps[0][:], sn, OP.mult)
                    nc.vector.tensor_tensor(to[:, s0:s0 + SC], t3[:], t4[:],
                                            OP.add)
                for sb4 in range(SC // 128):
                    jb = sc * (SC // 128) + sb4
                    vps = ps.tile([128, DLOC], F32, tag="pv0", name="vps")
                    _pe_break()
                    for db in range(NDB):
                        xsl = xt[:, db, 128 * sb4:128 * sb4 + 128]
                        _pe(nc.tensor.ldweights(xsl))
                        _pe(nc.tensor.matmul(
                            vps[:], xsl, wv[:, db, :],
                            start=(db == 0), stop=(db == NDB - 1)))
                    nc.vector.tensor_copy(
                        vp[:, jb, :, 0:64],
                        vps[:].rearrange("p (h c) -> p h c", h=HPC))

            def attn_itile(it):
                i0 = it * SC
                njb = i0 // 128 + 4
                pvs = [ps.tile([128, SC], F32, tag=f"pv{h}", name=f"pvt{h}")
                       for h in range(HPC)]
                pending = []
                for jb in range(njb):
                    _pe_break()
                    j0 = jb * 128
                    off = max(0, j0 - i0)
                    for hp in range(2):  # head pairs (0,1) and (2,3)
                        ss = ps.tile([128, 2, SC], F32, tag="st", bufs=2,
                                     name="ssjb")
                        for hh in range(2):
                            h = 2 * hp + hh
                            r0 = 32 * h
                            kesl = ke[r0:r0 + 32, j0:j0 + 128]
                            kosl = ko[r0:r0 + 32, j0:j0 + 128]
                            _pe(nc.tensor.ldweights(kesl,
                                                    tile_position=(r0, 0)))
                            _pe(nc.tensor.matmul(
                                ss[:, hh, off:SC], kesl,
                                qe[r0:r0 + 32, i0 + off:i0 + SC],
                                start=True, stop=False, tile_position=(r0, 0)))
                            _pe(nc.tensor.ldweights(kosl,
                                                    tile_position=(r0, 0)))
                            _pe(nc.tensor.matmul(
                                ss[:, hh, off:SC], kosl,
                                qo[r0:r0 + 32, i0 + off:i0 + SC],
                                start=False, stop=True, tile_position=(r0, 0)))
                        pt = sb.tile([128, 2, SC], F32R, tag="pt", bufs=4,
                                     name="pt")
                        nc.scalar.activation(pt[:, :, off:SC], ss[:, :, off:SC],
                                             AF.Exp, scale=0.125)
                        if j0 >= i0 and adet >= 3:
                            # diagonal: mask both heads' [128,128] windows
                            nc.vector.tensor_tensor(
                                pt[:, :, off:off + 128],
                                pt[:, :, off:off + 128], mtri2[:], OP.mult)
                        if adet >= 2:
                            def mk_pv(jb=jb, hp=hp, pt=pt, off=off):
                                for hh in range(2):
                                    h = 2 * hp + hh
                                    nc.tensor.matmul(
                                        pvs[h][:, off:SC],
                                        vp[:, jb, h, :], pt[:, hh, off:SC],
                                        start=(jb == 0),
                                        stop=(jb == njb - 1))
                            pending.append(mk_pv)
                    # emit the PREVIOUS j-block's PV after this block's scores
                    # so the PE stream never stalls on the current exp
                    while len(pending) > 2:
                        pending.pop(0)()
                for fn in pending:
                    fn()
                # normalize: ctx = pv[0:64] / r (r = pv[64:128], replicated)
                for h in range(HPC if adet >= 3 else 0):
                    rt = sb.tile([64, SC], F32, tag="t0", bufs=2, name="rt")
                    nc.scalar.activation(rt[:], pvs[h][64:128, :], AF.Copy)
                    rr = sb.tile([64, SC], F32, tag="t1", bufs=2, name="rr")
                    nc.vector.reciprocal_approx_fast(rr[:], rt[:])
                    cx = (cx0, cx1)[h // 2]
                    nc.vector.tensor_tensor(
                        cx[64 * (h % 2):64 * (h % 2) + 64, i0:i0 + SC],
                        pvs[h][0:64, :], rr[:], OP.mult)

            def wo_itile(it):
                i0 = it * SC
                for dt in range(NDB):
                    ops_ = ps.tile([128, SC], F32, tag="pv1", name="ops")
                    for dcb, cx in enumerate((cx0, cx1)):
                        nc.tensor.matmul(
                            ops_[:], wo[:, dcb, dt * 128:dt * 128 + 128],
                            cx[:, i0:i0 + SC],
                            start=(dcb == 0), stop=(dcb == 1))
                    ost = sb.tile([128, SC], F32, tag="ost", bufs=3, name="ost")
                    nc.vector.tensor_copy(ost[:], ops_[:])
                    oeng = (nc.sync, nc.sync, nc.gpsimd)[dt % 3]
                    oeng.dma_start(
                        d_out[dt * 128:(dt + 1) * 128, i0:i0 + SC], ost[:])

            w_ = "w" in phases
            if "p" in phases and "a" in phases:
                proj_chunk(0)
                proj_chunk(1)
                attn_itile(0)
                proj_chunk(2)
                if w_: wo_itile(0)
                attn_itile(1)
                proj_chunk(3)
                if w_: wo_itile(1)
                attn_itile(2)
                if w_: wo_itile(2)
                attn_itile(3)
                if w_: wo_itile(3)
            elif "p" in phases:
                for sc_ in range(NSC):
                    proj_chunk(sc_)
            elif "a" in phases:
                for it_ in range(NSC):
                    attn_itile(it_)
                if w_:
                    for it_ in range(NSC):
                        wo_itile(it_)
            elif w_:
                for it_ in range(NSC):
                    wo_itile(it_)

        if niter == 1:
            body()
        else:
            if "p" not in phases:
                # populate qe/ke/vp/cx once so the timed loop has real data
                saved = phases
                phases = "pa"
                body()
                phases = saved
            with tc.For_i(0, niter, 1) as i:
                body(i)
        ctx.close()
    nc.compile()
    return nc


def prep_inputs(x, pos_cos, pos_sin, Wq, Wk, Wv, Wo):
    """Host-side prep: per-core input dicts."""
    cosT = np.ascontiguousarray(np.tile(pos_cos.T, (4, 1)), np.float32)
    sinT = np.ascontiguousarray(np.tile(pos_sin.T, (4, 1)), np.float32)
    ones = np.ones((128, NJB * HPC * 64), np.float32)
    masktri = np.triu(np.ones((128, 128), np.float32))
    in_maps = []
    for c in range(N_CORES):
        b, g = c // 4, c % 4
        heads = [4 * g + h for h in range(HPC)]
        ecols = np.concatenate([64 * h + np.arange(0, 64, 2) for h in heads])
        ocols = np.concatenate([64 * h + np.arange(1, 64, 2) for h in heads])
        perm = np.concatenate([ecols, ocols])
        vcols = np.concatenate([64 * h + np.arange(64) for h in heads])
        in_maps.append({
            "xt": tobf16(x[b].T),
            "wq": tobf16(Wq[:, perm]),
            "wk": tobf16(Wk[:, perm]),
            "wv": tobf16(Wv[:, vcols]),
            "wo": preround(Wo[vcols, :]),
            "cosT": cosT, "sinT": sinT, "ones": ones, "masktri": masktri,
        })
    return in_maps


_NC_CACHE = {}


def get_program(niter=1, phases="paw", adet=3):
    key = (niter, phases, adet)
    if key not in _NC_CACHE:
        _NC_CACHE[key] = build_program(niter, phases, adet)
    return _NC_CACHE[key]


def run_on_cores(nc, in_maps, **kw):
    return run_bass_kernel_spmd(nc, in_maps, list(range(N_CORES)), **kw)


def kernel(x, pos_cos, pos_sin, Wq, Wk, Wv, Wo, bo):
    nc = get_program(1)
    in_maps = prep_inputs(x, pos_cos, pos_sin, Wq, Wk, Wv, Wo)
    res = run_on_cores(nc, in_maps)
    out = np.empty((B, S, D), np.float32)
    for b in range(B):
        acc = res.results[4 * b]["outp"].astype(np.float64)
        for g in range(1, 4):
            acc += res.results[4 * b + g]["outp"]
        out[b] = (acc.T + np.asarray(bo, np.float64)[None, :]).astype(np.float32)
    return out

